# revision 38
# baseline (speedup 1.0000x reference)
"""MultiHeadAttention (B=2, T=4096, H=6, hs=16, C=96) Bass kernel, 2 trn2 cores.

The device compute here is ~0.5ms/core; end-to-end time is dominated by the
axon tunnel RPC (~30-70ms floor) plus ~7-16ms/MB of host<->device transfer
(the previous 8-core version shipped ~53MB/call; this ships ~3.3MB). So the
design minimizes bytes moved per call:
  - core c <- batch c (2 cores, batch parallel). No duplicated activations.
  - x is shipped as bf16 (the kernel downcasts to bf16 for matmuls anyway),
    y is returned as bf16: ~1.8MB up + 1.6MB down per call total.
  - causal masks / identity / normalization constants are built on-device
    (affine_select + memset), so there are no constant input tensors at all.
  - exactly 3 input operands (x, stacked wqkv, packed wp+bp): the bass_exec
    custom call pays a per-operand cost, so fewer/larger args win (measured;
    note this is the OPPOSITE of plain-XLA jit calls on this plugin). The
    zero output placeholders are committed device arrays reused every call.

Attention runs in scores-transposed layout S^T[s, q] (s on partitions):
  S^T = matmul(lhsT=K^T[16, 128], rhs=Q^T[16, 512])    per head / s-block
  P   = exp(0.25 * S^T) via ScalarE (no max subtraction; scores are O(1))
  O^T[d, q] += matmul(lhsT=[V | 1 | 0..][128, 32], rhs=P) - the ones column
  gives the softmax denominator as row 16 of each head's O strip.
Heads are processed in pairs at partition strips 0/32. Queries are processed
in 8 contiguous supergroups of 512; s-blocks strictly above the diagonal are
skipped (44% of the work), and the 4 diagonal block shapes use constant
[128, 512]-doubled triangle masks built once with affine_select.
"""

import threading

import numpy as np
import ml_dtypes

import concourse.bass as bass
import concourse.mybir as mybir
from concourse import bacc
from concourse.tile import TileContext
from concourse.masks import make_identity

F32 = mybir.dt.float32
BF16 = mybir.dt.bfloat16

B, T, C = 2, 4096, 96
H, HS = 6, 16
NSB = T // 128   # 32 s-blocks
NSG = T // 512   # 8 query supergroups


def build_nc():
    nc = bacc.Bacc("TRN2", target_bir_lowering=False, debug=False,
                   enable_asserts=False)
    # Minimal arg count: the bass_exec custom call pays a per-operand cost
    # (measured: 1-chunk x beats 4- and 8-chunk splits, unlike pure XLA).
    # x is one tensor; Wq/Wk/Wv stack into wqkv; Wp and bp pack into wpb.
    x = nc.dram_tensor("x0", [T, C], BF16, kind="ExternalInput")
    wqkv = nc.dram_tensor("wqkv", [3 * H, C, HS], BF16, kind="ExternalInput")
    wpb = nc.dram_tensor("wpb", [C + 1, C], F32, kind="ExternalInput")
    y = nc.dram_tensor("y", [T, C], BF16, kind="ExternalOutput")

    with TileContext(nc) as tc:
        with (
            tc.tile_pool(name="one", bufs=1) as one,
            tc.tile_pool(name="stg", bufs=2) as stg,
            tc.tile_pool(name="pp", bufs=4) as pp,
            tc.tile_pool(name="wk2", bufs=2) as wk2,
            tc.tile_pool(name="sps", bufs=2, space="PSUM") as sps,
            tc.tile_pool(name="ops", bufs=2, space="PSUM") as ops,
        ):
            ident = one.tile([128, 128], F32, tag="ident")
            make_identity(nc, ident)
            identb = one.tile([128, 128], BF16, tag="identb")
            make_identity(nc, identb)

            # diagonal-block causal masks, built on device: within a 512-wide
            # query supergroup, s-block at diagonal offset d keeps (q >= s),
            # i.e. col >= 128*d + row. Doubled along free dim for head pairs.
            msk = []
            for d in range(4):
                mf = stg.tile([128, 1024], F32, tag="mstg")
                nc.gpsimd.memset(mf, 1.0)
                for l in range(2):
                    nc.gpsimd.affine_select(
                        out=mf[:, 512 * l:512 * (l + 1)],
                        in_=mf[:, 512 * l:512 * (l + 1)],
                        compare_op=mybir.AluOpType.is_ge,
                        fill=0.0, base=-128 * d,
                        pattern=[[1, 512]], channel_multiplier=-1)
                mb = one.tile([128, 1024], BF16, tag=f"msk{d}")
                nc.vector.tensor_copy(mb, mf)
                msk.append(mb)

            # normalization constants (Em routes each head's denominator row
            # onto that head's 16 output rows; urow puts 1.0 in unused rows).
            # The denominator lives at strip row 0 (not 16) so Em's writes
            # start at partitions 0/32 — engine partition starts must be
            # 32-aligned.
            Em = one.tile([64, 64], F32, tag="Em")
            nc.gpsimd.memset(Em, 0.0)
            urow = one.tile([1, 64], F32, tag="urow")
            nc.gpsimd.memset(urow, 0.0)
            for l in range(2):
                nc.gpsimd.memset(Em[32 * l:32 * l + 1,
                                    32 * l:32 * l + HS + 1], 1.0)
                nc.gpsimd.memset(urow[0:1, 32 * l + HS + 1:32 * l + 32], 1.0)
            ones_r = one.tile([1, 512], F32, tag="ones")
            nc.gpsimd.memset(ones_r, 1.0)

            # padded per-pair projection weights: cols 32l+d <- W[2gg+l][:, d]
            # (wqkv rows: 0..5 = Wq heads, 6..11 = Wk heads, 12..17 = Wv)
            wq_pad, wk_pad = [], []
            for gg in range(3):
                for name, off, dst in (("q", 0, wq_pad), ("k", H, wk_pad)):
                    t = one.tile([C, 64], BF16, tag=f"w{name}{gg}")
                    nc.gpsimd.memset(t, 0.0)
                    for l in range(2):
                        nc.sync.dma_start(out=t[:, 32 * l:32 * l + HS],
                                          in_=wqkv[off + 2 * gg + l])
                    dst.append(t)
            wv_cat = one.tile([C, C], BF16, tag="wvcat")
            for h in range(H):
                nc.sync.dma_start(out=wv_cat[:, HS * h:HS * h + HS],
                                  in_=wqkv[2 * H + h])
            # Wp^T padded per pair: rows 32l+1+d <- Wp[:, 16(2gg+l)+d]
            # (row 32l is the denominator row and stays zero)
            wp_pad = []
            for gg in range(3):
                s = stg.tile([C, 64], F32, tag="wstg")
                nc.gpsimd.memset(s, 0.0)
                for l in range(2):
                    h = 2 * gg + l
                    nc.sync.dma_start(out=s[:, 32 * l + 1:32 * l + 1 + HS],
                                      in_=wpb[0:C, HS * h:HS * h + HS])
                psw = sps.tile([64, C], F32, tag="S")
                nc.tensor.transpose(psw, s, ident[:C, :C])
                t = one.tile([64, C], F32, tag=f"wp{gg}")
                nc.vector.tensor_copy(t, psw)
                wp_pad.append(t)
            bp_b = one.tile([128, C], F32, tag="bpb")
            bpap = wpb[C]
            nc.sync.dma_start(out=bp_b, in_=bass.AP(
                tensor=bpap.tensor, offset=bpap.offset, ap=[[0, 128]] + list(bpap.ap)))

            # ---- X^T ----
            xT = one.tile([C, T], BF16, tag="xT")
            for tb in range(NSB):
                xt = stg.tile([128, C], BF16, tag="xt")
                nc.sync.dma_start(out=xt, in_=x[128 * tb:128 * (tb + 1), :])
                xtf = stg.tile([128, C], F32, tag="xtf")
                nc.vector.tensor_copy(xtf, xt)
                ps = sps.tile([C, 128], F32, tag="S")
                nc.tensor.transpose(ps, xtf, ident)
                nc.vector.tensor_copy(xT[:, 128 * tb:128 * (tb + 1)], ps)

            # ---- K^T, Q^T, V_store ----
            kT, qT = [], []
            for gg in range(3):
                for wpad, dst, nm in ((wk_pad, kT, "k"), (wq_pad, qT, "q")):
                    t = one.tile([64, T], BF16, tag=f"{nm}T{gg}")
                    for cc in range(T // 512):
                        ps = sps.tile([64, 512], F32, tag="S")
                        nc.tensor.matmul(ps, wpad[gg],
                                         xT[:, 512 * cc:512 * (cc + 1)],
                                         start=True, stop=True)
                        nc.vector.tensor_copy(t[:, 512 * cc:512 * (cc + 1)], ps)
                    dst.append(t)
            # vst col 0 is the ones column (softmax denominator); V in 1..16
            vst = one.tile([128, NSB, H, 32], BF16, tag="vst")
            nc.gpsimd.memset(vst, 0.0)
            for h in range(H):
                nc.gpsimd.memset(vst[:, :, h, 0:1], 1.0)
            for tb in range(NSB):
                ps = sps.tile([128, C], F32, tag="S")
                nc.tensor.matmul(ps, xT[:, 128 * tb:128 * (tb + 1)], wv_cat,
                                 start=True, stop=True)
                nc.vector.tensor_copy(
                    vst[:, tb, :, 1:17],
                    ps.rearrange("p (h d) -> p h d", d=HS))

            # ---- attention + output projection, per query supergroup ----
            for sg in range(NSG):
                o_fin = []
                n_sb = 4 * sg + 4
                for gg in range(3):
                    o_ps = [ops.tile([32, 512], F32, tag=f"O{l}", name=f"ops{l}")
                            for l in range(2)]
                    for sb in range(n_sb):
                        s_ps = sps.tile([128, 1024], F32, tag="S")
                        for l in range(2):
                            nc.tensor.matmul(
                                s_ps[:, 512 * l:512 * (l + 1)],
                                kT[gg][32 * l:32 * l + HS, 128 * sb:128 * (sb + 1)],
                                qT[gg][32 * l:32 * l + HS, 512 * sg:512 * (sg + 1)],
                                start=True, stop=True)
                        p = pp.tile([128, 1024], BF16, tag="P")
                        nc.scalar.activation(p, s_ps,
                                             mybir.ActivationFunctionType.Exp,
                                             scale=0.25)
                        d = sb - 4 * sg
                        if d >= 0:
                            nc.vector.tensor_mul(p, p, msk[d])
                        for l in range(2):
                            nc.tensor.matmul(
                                o_ps[l],
                                vst[:, sb, 2 * gg + l, :],
                                p[:, 512 * l:512 * (l + 1)],
                                start=(sb == 0), stop=(sb == n_sb - 1))
                    o_nrm = wk2.tile([64, 512], F32, tag=f"onrm{gg}")
                    for l in range(2):
                        nc.vector.tensor_copy(o_nrm[32 * l:32 * l + 32, :], o_ps[l])
                    r_ps = sps.tile([64, 512], F32, tag="S")
                    nc.tensor.matmul(r_ps, Em, o_nrm, start=True, stop=False)
                    nc.tensor.matmul(r_ps, urow, ones_r, start=False, stop=True)
                    r_sb = wk2.tile([64, 512], F32, tag="rsb")
                    nc.vector.reciprocal(r_sb, r_ps)
                    of = wk2.tile([64, 512], F32, tag=f"of{gg}")
                    nc.vector.tensor_mul(of, o_nrm, r_sb)
                    o_fin.append(of)

                for st in range(4):
                    y_ps = ops.tile([128, C], F32, tag="O0")
                    for gg in range(3):
                        nc.tensor.matmul(
                            y_ps, o_fin[gg][:, 128 * st:128 * (st + 1)],
                            wp_pad[gg], start=(gg == 0), stop=(gg == 2))
                    y_sb = wk2.tile([128, C], BF16, tag="ysb")
                    nc.vector.tensor_add(y_sb, y_ps, bp_b)
                    nc.sync.dma_start(
                        out=y[512 * sg + 128 * st:512 * sg + 128 * (st + 1), :],
                        in_=y_sb)
    nc.finalize()
    return nc


_NC_CACHE = {}
_NC_LOCK = threading.Lock()
N_CORES = 2


def _fast_runner(nc):
    """Persistent shard_map jit over 2 cores (mirrors run_bass_via_pjrt, but
    reusable across calls so we only pay jax dispatch per call)."""
    import jax
    from jax.sharding import Mesh, PartitionSpec
    from jax.experimental.shard_map import shard_map
    from concourse import bass2jax
    bass2jax.install_neuronx_cc_hook()
    in_names, out_names, out_avals, zero_outs = [], [], [], []
    in_specs_sd = []
    for alloc in nc.m.functions[0].allocations:
        if not isinstance(alloc, mybir.MemoryLocationSet):
            continue
        name = alloc.memorylocations[0].name
        if alloc.kind == "ExternalInput":
            if nc.partition_id_tensor is None or name != nc.partition_id_tensor.name:
                in_names.append(name)
                in_specs_sd.append((tuple(alloc.tensor_shape),
                                    mybir.dt.np(alloc.dtype)))
        elif alloc.kind == "ExternalOutput":
            out_names.append(name)
            shape = tuple(alloc.tensor_shape)
            dtype = mybir.dt.np(alloc.dtype)
            out_avals.append(jax.core.ShapedArray(shape, dtype))
            zero_outs.append(np.zeros(shape, dtype))
    n_params = len(in_names)
    all_names = in_names + out_names
    if nc.partition_id_tensor is not None:
        all_names = all_names + [nc.partition_id_tensor.name]

    def _body(*args):
        ops_ = list(args)
        if nc.partition_id_tensor is not None:
            ops_.append(bass2jax.partition_id_tensor())
        return tuple(bass2jax._bass_exec_p.bind(
            *ops_, out_avals=tuple(out_avals), in_names=tuple(all_names),
            out_names=tuple(out_names), lowering_input_output_aliases=(),
            sim_require_finite=True, sim_require_nnan=True, nc=nc))

    devices = jax.devices()[:N_CORES]
    mesh = Mesh(np.asarray(devices), ("core",))
    nin = n_params + len(out_names)

    def make_jit():
        return jax.jit(shard_map(_body, mesh=mesh,
                                 in_specs=(PartitionSpec("core",),) * nin,
                                 out_specs=(PartitionSpec("core"),) * len(out_names),
                                 check_rep=False), keep_unused=True)

    # fast_dispatch_compile suppresses bass_effect so calls go through the
    # C++ dispatch fast path instead of the Python effects-token machinery.
    sh = jax.sharding.NamedSharding(mesh, PartitionSpec("core"))
    try:
        example = [jax.ShapeDtypeStruct((N_CORES * s[0], *s[1:]), dt, sharding=sh)
                   for s, dt in in_specs_sd]
        example += [jax.ShapeDtypeStruct((N_CORES * z.shape[0], *z.shape[1:]),
                                         z.dtype, sharding=sh)
                    for z in zero_outs]
        sharded = bass2jax.fast_dispatch_compile(
            lambda: make_jit().lower(*example).compile())
    except Exception:
        sharded = make_jit()

    # output placeholder buffers live on device permanently — shipping fresh
    # zeros every call would cost ~1.6MB of tunnel traffic for nothing.
    zero_cache = [jax.device_put(
        np.zeros((N_CORES * z.shape[0], *z.shape[1:]), z.dtype), sh)
        for z in zero_outs]
    yidx = out_names.index("y")

    class Runner:
        def device_put(self, concat_in):
            # block: dispatching an exec whose input transfers are still in
            # flight can wedge the exec unit (NRT_EXEC_UNIT_UNRECOVERABLE)
            dev = {nm: jax.device_put(concat_in[nm], sh) for nm in in_names}
            jax.block_until_ready(list(dev.values()))
            return dev

        def dispatch(self, in_map):
            return sharded(*[in_map[nm] for nm in in_names], *zero_cache)

        def fetch_y(self, outs):
            return np.asarray(outs[yidx])

        def run(self, in_map):
            return self.fetch_y(self.dispatch(in_map))

    return Runner()


def kernel(x, Wq, Wk, Wv, Wp, bp):
    x = np.asarray(x, np.float32)
    with _NC_LOCK:
        if "nc" not in _NC_CACHE:
            _NC_CACHE["nc"] = build_nc()
    nc = _NC_CACHE["nc"]

    BF = ml_dtypes.bfloat16
    raws = [x, np.asarray(Wq, np.float32), np.asarray(Wk, np.float32),
            np.asarray(Wv, np.float32), np.asarray(Wp, np.float32),
            np.asarray(bp, np.float32)]

    def per_core():
        qkv = np.concatenate([raws[1], raws[2], raws[3]], axis=0).astype(BF)
        wpb = np.concatenate([raws[4], raws[5][None, :]], axis=0)
        return qkv, wpb

    def build_concat():
        xbf = np.ascontiguousarray(x).astype(BF)   # [B, T, C]
        qkv, wpb = per_core()
        return {
            "x0": xbf.reshape(B * T, C),
            "wqkv": np.concatenate([qkv] * N_CORES, axis=0),
            "wpb": np.concatenate([wpb] * N_CORES, axis=0),
        }

    try:
        if "runner" not in _NC_CACHE:
            _NC_CACHE["runner"] = _fast_runner(nc)
        # np args + committed zero placeholders is the fastest path on this
        # plugin: the whole upload->exec->fetch pipeline completes in one
        # pipelined RPC exchange. (Committed input refs, standalone fetches,
        # and cross-call speculative dispatch all measured slower.)
        yflat = _NC_CACHE["runner"].run(build_concat())
    except Exception:
        from concourse import bass_utils
        xbf = np.ascontiguousarray(x).astype(BF)
        qkv, wpb = per_core()
        in_maps = [{"x0": xbf[c], "wqkv": qkv, "wpb": wpb}
                   for c in range(N_CORES)]
        results = bass_utils.run_bass_kernel_spmd(
            nc, in_maps, core_ids=list(range(N_CORES))).results
        yflat = np.concatenate([results[c]["y"] for c in range(N_CORES)], axis=0)
    return np.asarray(yflat).astype(np.float32).reshape(B, T, C)
